# revision 44
# baseline (speedup 1.0000x reference)
"""Trainium2 Bass kernel for nn_AttentionHead (B=8, S=4096, D=128).

Sharding: data-parallel over the batch dim — 1 batch element per NeuronCore,
8 cores, SPMD (same NEFF, different x slice), weights replicated. No
collectives.

Per-core pipeline (S=4096 seq, D=128 head dim, all-on-chip, f16 compute
with f32 PSUM accumulation; fro rel err vs fp32 reference ~5e-4):
  1. x [4096,128] f32 -> cast-load f16 (SWDGE cast DMA) -> chunked
     DMA-xbar-transposes -> xT [d, s] f16 (chunked so projections start
     after the first quarter)
  2. q/k/v projections: matmul(lhsT=xT s-tile, rhs=W^T) -> PSUM f32.
     Two passes: (A) per tile stage raw q/k to SBUF f16 (ACT) with
     bn_stats/bn_aggr reading the staged f16 (DVE), v copies split
     ACT/DVE by parity (pass A is otherwise DVE-bound); then one batched
     rsqrt for all 64 rows via
     exp(-0.5*ln(var+eps)) — Ln/Exp share one ACT table set with the
     attention Exp, so the whole kernel needs ~3 table loads (a per-tile
     Sqrt thrashes 3.6us table reloads against the attention Exps);
     (B) apply LN on DVE (per-partition scale + broadcast bias) into
     [s,t,h] staging, one batched DMA-xbar-transpose per tensor to [h,t,s],
     then a single big ACT op folds LN weight/bias (per-partition scalars
     after the transpose). ALL xbar transposes go on the sync HWDGE ring:
     concurrent transposes on the sync+scalar rings corrupt data on HW.
  3. attention with TRANSPOSED scores (scT[j,i] = k_j . q_i), looping over
     1024-wide query column blocks (i-quarters):
     - per key tile jt (128 keys): scT = matmul(lhsT=kT[:,jt,:],
       rhs=qT columns) into a 2-bank PSUM tile; exp via ACT off PSUM with
       scale=1/sqrt(D) directly into eT [j, i] f16 — already the layout PV
       needs, so the 32MB of per-i-tile exp xbar transposes of the old
       scheme disappear entirely. No max subtraction: scores are ~N(0,1)
       (LN'd q,k), exp stays in f16 range.
     - PV accumulates outT[h, i] += v16[:,jt,:]^T @ eT over all 32 key
       tiles (v is the stationary matmul operand, eT streams 512-col
       chunks), and a parallel ones-column matmul accumulates the softmax
       denominators rs[1, i] — 6 wide 512-col matmuls per key tile instead
       of the old scheme's 32 tiny 129-col PV matmuls per query tile.
     - drain: outT -> f16 SBUF (DVE), one xbar transpose per i-quarter
       back to [i, h]; rs row is PE-transposed (matmul-with-identity, 32
       [1,128]->[128,1] blocks) into per-partition scalars, reciprocal on
       DVE, then per i-tile normalize (tensor_scalar) and DMA out.
     Software pipeline: QK[jt] is emitted before PV[jt-1] so the PE runs
     the ACT-feeding scores matmul first; sc PSUM pool is double-buffered,
     eT triple-buffered.

All SBUF pools stay open for the whole kernel (no SBUF slot reuse across
phases): SBUF-space reuse attaches release waits to the DMAs that load into
recycled space, and walrus rejects DMAs with more than a couple of sync
waits ("Too many sync wait commands"). Only PSUM pools are scoped.
"""

import math

import numpy as np

from concourse import bacc
import concourse.mybir as mybir
import concourse.tile as tile
from concourse.bass_utils import run_bass_kernel_spmd


F16 = mybir.dt.float16
F32 = mybir.dt.float32
AF = mybir.ActivationFunctionType
ALU = mybir.AluOpType

B, S, D = 8, 4096, 128
P = 128
NT = S // P  # 32 s-tiles
EPS = 1e-5
ISQRT_D = 1.0 / math.sqrt(D)
N_CORES = 8
_ABLATE = set()  # timing-ablation flags, empty in production
IQ = 4  # query-column blocks in the attention loop (1024 cols = 2 banks)


def _ln_param_to_sbuf(nc, pool, dram_ap, tag):
    t = pool.tile([P, 1], F32, tag=tag)
    nc.sync.dma_start(t, dram_ap[:, None])
    return t


def _build_attention(tc, out_d, x_d, w_d, ln_d):
    """Emit the single-core attention program.

    out_d: [S, D] f32 output AP.  x_d: [S, D] f32 input AP.
    w_d: dict q/k/v -> [D, D] f32 weight AP (torch Linear layout: out = x @ W^T).
    ln_d: dict qw/qb/kw/kb -> [D] f32 LN param APs.
    """
    nc = tc.nc

    with (
        tc.tile_pool(name="const", bufs=1) as const,
        tc.tile_pool(name="big", bufs=1) as big,
        tc.tile_pool(name="wtmp", bufs=3) as wtmp,
        tc.tile_pool(name="xload", bufs=1) as xload,
        tc.tile_pool(name="stat", bufs=6) as stat,
        tc.tile_pool(name="attn", bufs=4) as attn,
        tc.tile_pool(name="stage", bufs=2) as stage,
        tc.tile_pool(name="small", bufs=4) as small,
    ):
        # --- weights: load [h,d] f32, cast f16, DMA-xbar-transpose -> W^T f16
        WT = {}
        for name in ("q", "k", "v"):
            w32 = wtmp.tile([P, P], F32, tag=f"w32_{name}")
            nc.sync.dma_start(w32, w_d[name])
            w16 = wtmp.tile([P, P], F16, tag=f"w16_{name}")
            nc.vector.tensor_copy(w16, w32)
            wt = const.tile([P, P], F16, tag=f"wt_{name}")
            nc.sync.dma_start_transpose(wt, w16)
            WT[name] = wt

        # --- x load (cast f32->f16 in the SWDGE DMA), one batched
        # DMA-xbar-transpose: xT[d, t, s] = x16[s, t*128+d]
        xT = big.tile([P, NT, P], F16, tag="xT")  # [d, t, s%128]
        x16 = xload.tile([P, NT, P], F16)  # [s%128, t, d]
        _ab = _ABLATE
        # load and transpose in quarters so the first projections start
        # as soon as the first 8 s-tiles are resident
        x_r = x_d.rearrange("(t p) d -> p t d", p=P)
        x16f = x16.rearrange("p t d -> p (t d)")
        for c in range(4):
            nc.gpsimd.dma_start(x16[:, c * 8:(c + 1) * 8, :],
                                x_r[:, c * 8:(c + 1) * 8, :])
            nc.sync.dma_start_transpose(
                xT[:, c * 8:(c + 1) * 8, :],
                x16f[:, c * 8 * P:(c + 1) * 8 * P])

        # LN params loaded after the x/weight ring traffic: they head-of-line
        # block the sync ring for ~2us if issued first, and aren't needed
        # until pass B
        qnw = _ln_param_to_sbuf(nc, const, ln_d["qw"], "qnw")
        qnb = _ln_param_to_sbuf(nc, const, ln_d["qb"], "qnb")
        knw = _ln_param_to_sbuf(nc, const, ln_d["kw"], "knw")
        knb = _ln_param_to_sbuf(nc, const, ln_d["kb"], "knb")

        # --- projections + layernorm -> qT, kT [h, s] f16; v [s, h] f16
        # Two passes: (A) project q/k/v, stage raw q/k + bn stats; then ONE
        # batched rsqrt for all 64 (tile, tensor) rows via exp(-0.5*ln(v+eps))
        # (a single Ln + Exp keeps ACT table switching to ~2 loads; per-tile
        # Sqrt thrashes table sets against the attention Exp ops); (B) apply
        # LN, transpose, fold ln weight/bias.
        qT = big.tile([P, NT, P], F16, tag="qT")
        kT = big.tile([P, NT, P], F16, tag="kT")
        v16 = big.tile([P, NT, P], F16, tag="v16")
        raw = big.tile([P, 2 * NT, P], F16, tag="raw")  # pre-LN q/k
        mvall = big.tile([P, 2 * NT, 2], F32, tag="mvall")  # (mean, var)
        rstd_all = big.tile([P, 2 * NT], F32, tag="rstd_all")
        nmr_all = big.tile([P, 2 * NT], F32, tag="nmr_all")
        s1q = big.tile([P, NT, P], F16, tag="s1q")
        s1k = big.tile([P, NT, P], F16, tag="s1k")
        qT_pre = big.tile([P, NT, P], F16, tag="qT_pre")
        kT_pre = big.tile([P, NT, P], F16, tag="kT_pre")
        specs = {
            "q": (s1q, qT_pre, qT, qnw, qnb, 0),
            "k": (s1k, kT_pre, kT, knw, knb, 1),
        }

        def emit_ln_half(name, hh):
            # LN apply (DVE) -> xbar transpose (sync ring) -> wb fold (ACT)
            s1all, pre, Tdst, wsb, bsb, koff = specs[name]
            for t in range(hh * 16, (hh + 1) * 16):
                idx = 2 * t + koff
                nc.vector.scalar_tensor_tensor(
                    s1all[:, t, :], in0=raw[:, idx, :],
                    scalar=rstd_all[:, idx:idx + 1],
                    in1=nmr_all[:, idx:idx + 1].to_broadcast([P, P]),
                    op0=ALU.mult, op1=ALU.add)
            s1f = s1all.rearrange("p t h -> p (t h)")
            T2 = Tdst.rearrange("h t s -> h (t s)")
            P2 = pre.rearrange("h t s -> h (t s)")
            nc.sync.dma_start_transpose(
                pre[:, hh * 16:(hh + 1) * 16, :],
                s1f[:, hh * 16 * P:(hh + 1) * 16 * P])
            nc.scalar.activation(
                T2[:, hh * 16 * P:(hh + 1) * 16 * P],
                P2[:, hh * 16 * P:(hh + 1) * 16 * P],
                AF.Identity, scale=wsb, bias=bsb)

        # Projection + LN run in HALVES so half 0's rsqrt/apply/transpose/
        # fold chain overlaps half 1's projections: attention then starts
        # right after half 1's LN instead of serializing the whole chain
        # behind all 32 projections. Half order q,k then k,q: attention
        # consumes kT progressively but needs qT half 0 for its first
        # query block.
        with tc.tile_pool(name="pps", bufs=2, space="PSUM") as pps:
            for hh in range(2 if "ph12" not in _ab else 0):
                for t in range(hh * 16, (hh + 1) * 16):
                    for k, name in enumerate(("q", "k", "v")):
                        ps = pps.tile([P, P], F32, tag=f"p_{name}")
                        nc.tensor.matmul(ps, lhsT=xT[:, t, :], rhs=WT[name],
                                         start=True, stop=True)
                        if name == "v":
                            # split PSUM evacuations across ACT/DVE: pass A
                            # is otherwise DVE-bound, ACT has headroom
                            # (gpsimd tensor_copy from PSUM fails to lower)
                            if t % 2 == 0:
                                nc.vector.tensor_copy(v16[:, t, :], ps)
                            else:
                                nc.scalar.activation(v16[:, t, :], ps,
                                                     AF.Copy)
                        else:
                            idx = 2 * t + k
                            nc.scalar.activation(raw[:, idx, :], ps, AF.Copy)
                            # stats from the staged f16 raw (SBUF read is
                            # cheaper on DVE than PSUM, and the stats then
                            # describe exactly what pass B normalizes)
                            st = stat.tile([P, 6], F32, tag="st")
                            nc.vector.bn_stats(st, raw[:, idx, :])
                            nc.vector.bn_aggr(mvall[:, idx, :], st)
                # rsqrt(v) = exp(-0.5 * ln(v)) batched over this half's 32
                # (tile, tensor) rows; Ln/Exp share one ACT table set with
                # the attention Exp so this costs no extra table loads
                sl = slice(hh * NT, (hh + 1) * NT)
                vare = stat.tile([P, NT], F32, tag=f"vare{hh}")
                nc.vector.tensor_scalar_add(vare, mvall[:, sl, 1], EPS)
                nc.scalar.activation(rstd_all[:, sl], vare, AF.Ln)
                nc.scalar.activation(rstd_all[:, sl], rstd_all[:, sl],
                                     AF.Exp, scale=-0.5)
                nc.vector.scalar_tensor_tensor(
                    nmr_all[:, sl], in0=mvall[:, sl, 0], scalar=-1.0,
                    in1=rstd_all[:, sl], op0=ALU.mult, op1=ALU.mult)
                for name in (("q", "k") if hh == 0 else ("k", "q")):
                    emit_ln_half(name, hh)

        # --- attention (transposed scores: scT[j, i] = k_j . q_i)
        qT2 = qT.rearrange("h t s -> h (t s)")
        ones11 = const.tile([1, 1], F32, tag="ones11")
        nc.vector.memset(ones11, 1.0)
        ones16 = const.tile([P, 1], F16, tag="ones16")
        nc.vector.memset(ones16, 1.0)
        rs_sb = big.tile([1, S], F32, tag="rs_sb")  # softmax denominators
        rsrT = big.tile([P, NT], F32, tag="rsrT")  # 1/rs, [i%128, it]
        oT = big.tile([P, NT, P], F16, tag="oT")  # [i%128, it, h] unnormalized

        IQW = S // IQ  # query columns per block
        TPQ = NT // IQ  # i-tiles per block
        with (
            tc.tile_pool(name="scps", bufs=3, space="PSUM") as scps,
            tc.tile_pool(name="outps", bufs=1, space="PSUM") as outps,
            tc.tile_pool(name="rcp", bufs=2) as rcp,
        ):
            for iq in range(IQ):
                outp = outps.tile([P, IQW], F32, tag="outp")
                # rowsum partials accumulate on DVE (f16 ping-pong): frees
                # 2 PE matmuls per key tile AND the rs PSUM banks, which
                # buy the 3-deep scores pool that hides the QK->exp->free
                # semaphore latency. Partitions collapse via one
                # ones-matmul per block at the end.
                raccA0 = rcp.tile([P, IQW], F16, tag="raccA0")
                raccA1 = rcp.tile([P, IQW], F16, tag="raccA1")
                raccB0 = rcp.tile([P, IQW], F16, tag="raccB0")
                raccB1 = rcp.tile([P, IQW], F16, tag="raccB1")
                # two independent accumulate chains (even/odd key tiles):
                # a single chain is a serial 1.13us-latency dependency and
                # binds the whole loop; two chains make it throughput-bound
                racc = {0: [raccA0, raccA1], 1: [raccB0, raccB1]}

                def emit_qk_exp(jt, iq=iq):
                    sc = scps.tile([P, IQW], F32, tag="sc")
                    if "qk" not in _ab:
                        for h in range(IQW // 512):
                            nc.tensor.matmul(
                                sc[:, h * 512:(h + 1) * 512],
                                lhsT=kT[:, jt, :],
                                rhs=qT2[:, iq * IQW + h * 512:
                                        iq * IQW + (h + 1) * 512],
                                start=True, stop=True)
                    eT = attn.tile([P, IQW], F16, tag="eT")  # [j%128, i]
                    if "exp" not in _ab:
                        nc.scalar.activation(eT, sc, AF.Exp, scale=ISQRT_D)
                    return eT

                def emit_pv(jt, eT, outp=outp, racc=racc):
                    first, last = jt == 0, jt == NT - 1
                    if "pv" not in _ab:
                        for h in range(IQW // 512):
                            nc.tensor.matmul(
                                outp[:, h * 512:(h + 1) * 512],
                                lhsT=v16[:, jt, :],
                                rhs=eT[:, h * 512:(h + 1) * 512],
                                start=first, stop=last)
                    chain, m = racc[jt % 2], jt // 2
                    if m == 0:
                        nc.vector.tensor_copy(chain[0], eT)
                    else:
                        nc.vector.scalar_tensor_tensor(
                            chain[m % 2], in0=chain[(m + 1) % 2],
                            scalar=1.0, in1=eT,
                            op0=ALU.mult, op1=ALU.add)

                # sw-pipeline: QK[jt] ahead of PV[jt-1] in the PE stream so
                # the ACT-feeding scores matmul never queues behind PV
                prev = None
                for jt in range(NT):
                    cur = emit_qk_exp(jt)
                    if prev is not None:
                        emit_pv(jt - 1, prev)
                    prev = cur
                emit_pv(NT - 1, prev)

                # drain this query block: outT -> f16, xbar back to [i, h];
                # rs row -> per-partition scalars via k=1 matmuls
                # (out[m, 0] = rs_row[0, m] * ones[0, 0]) reusing the rsps
                # pool slot, then normalize + write out this block — keeps
                # the output DMA overlapped with the next block's attention
                o16 = stage.tile([P, IQW], F16, tag="o16")
                # ACT copy: DVE is the binding engine in the attention loop
                nc.scalar.activation(o16, outp, AF.Copy)
                nc.sync.dma_start_transpose(
                    oT[:, iq * TPQ:(iq + 1) * TPQ, :], o16)
                # rs partials -> [1, IQW] via ones-matmul, riding the outp
                # PSUM slot (free once o16 is drained)
                # combine the two chains (last write of each is slot 1:
                # m=15 for both), then collapse partitions via ones-matmul
                nc.vector.scalar_tensor_tensor(
                    racc[0][0], in0=racc[0][1], scalar=1.0, in1=racc[1][1],
                    op0=ALU.mult, op1=ALU.add)
                rsp = outps.tile([1, IQW], F32, tag="outp")
                for h in range(IQW // 512):
                    nc.tensor.matmul(rsp[:, h * 512:(h + 1) * 512],
                                     lhsT=ones16,
                                     rhs=racc[0][0][:, h * 512:(h + 1) * 512],
                                     start=True, stop=True)
                nc.vector.tensor_copy(rs_sb[:, iq * IQW:(iq + 1) * IQW], rsp)
                rsT_ps = outps.tile([P, TPQ], F32, tag="outp")
                for tt in range(TPQ):
                    t = iq * TPQ + tt
                    nc.tensor.matmul(rsT_ps[:, tt:tt + 1],
                                     lhsT=rs_sb[:, t * P:(t + 1) * P],
                                     rhs=ones11, start=True, stop=True)
                nc.vector.reciprocal(rsrT[:, iq * TPQ:(iq + 1) * TPQ], rsT_ps)
                for tt in range(TPQ):
                    t = iq * TPQ + tt
                    osb = small.tile([P, P], F32, tag="osb")
                    nc.vector.tensor_scalar_mul(osb, oT[:, t, :],
                                                rsrT[:, t:t + 1])
                    # alternate rings so the final block's drain isn't
                    # serialized on one DGE ring
                    ring = nc.gpsimd if tt % 2 == 0 else nc.sync
                    ring.dma_start(out_d[t * P:(t + 1) * P, :], osb)


_NC_CACHE = None


def _build():
    global _NC_CACHE
    if _NC_CACHE is not None:
        return _NC_CACHE
    nc = bacc.Bacc("TRN2", target_bir_lowering=False, debug=False)
    x = nc.dram_tensor("x", [S, D], F32, kind="ExternalInput").ap()
    wq = nc.dram_tensor("Wq", [D, D], F32, kind="ExternalInput").ap()
    wk = nc.dram_tensor("Wk", [D, D], F32, kind="ExternalInput").ap()
    wv = nc.dram_tensor("Wv", [D, D], F32, kind="ExternalInput").ap()
    qn_w = nc.dram_tensor("qn_w", [D], F32, kind="ExternalInput").ap()
    qn_b = nc.dram_tensor("qn_b", [D], F32, kind="ExternalInput").ap()
    kn_w = nc.dram_tensor("kn_w", [D], F32, kind="ExternalInput").ap()
    kn_b = nc.dram_tensor("kn_b", [D], F32, kind="ExternalInput").ap()
    out = nc.dram_tensor("out", [S, D], F32, kind="ExternalOutput").ap()
    with tile.TileContext(nc) as tc:
        _build_attention(
            tc, out, x,
            {"q": wq, "k": wk, "v": wv},
            {"qw": qn_w, "qb": qn_b, "kw": kn_w, "kb": kn_b},
        )
    nc.compile()
    _NC_CACHE = nc
    return nc


def kernel(x, Wq, Wk, Wv, qn_w, qn_b, kn_w, kn_b, _run_kwargs=None):
    nc = _build()
    x = np.asarray(x, dtype=np.float32)
    shared = {
        "Wq": np.ascontiguousarray(np.asarray(Wq, np.float32)),
        "Wk": np.ascontiguousarray(np.asarray(Wk, np.float32)),
        "Wv": np.ascontiguousarray(np.asarray(Wv, np.float32)),
        "qn_w": np.ascontiguousarray(np.asarray(qn_w, np.float32)),
        "qn_b": np.ascontiguousarray(np.asarray(qn_b, np.float32)),
        "kn_w": np.ascontiguousarray(np.asarray(kn_w, np.float32)),
        "kn_b": np.ascontiguousarray(np.asarray(kn_b, np.float32)),
    }
    in_maps = [
        {"x": np.ascontiguousarray(x[b]), **shared} for b in range(B)
    ]
    res = run_bass_kernel_spmd(nc, in_maps, core_ids=list(range(N_CORES)),
                               **(_run_kwargs or {}))
    out = np.stack([res.results[b]["out"] for b in range(B)], axis=0)
    if _run_kwargs:
        kernel.last_results = res
    return out.astype(np.float32)



# revision 45
# speedup vs baseline: 1.2947x; 1.2947x over previous
"""Trainium2 Bass kernel for nn_AttentionHead (B=8, S=4096, D=128).

Sharding: data-parallel over the batch dim — 1 batch element per NeuronCore,
8 cores, SPMD (same NEFF, different x slice), weights replicated. No
collectives.

Per-core pipeline (S=4096 seq, D=128 head dim, all-on-chip, f16 compute
with f32 PSUM accumulation; fro rel err vs fp32 reference ~5e-4):
  1. x [4096,128] f32 -> cast-load f16 (SWDGE cast DMA) -> chunked
     DMA-xbar-transposes -> xT [d, s] f16 (chunked so projections start
     after the first quarter)
  2. q/k/v projections: matmul(lhsT=xT s-tile, rhs=W^T) -> PSUM f32.
     Two passes: (A) per tile stage raw q/k to SBUF f16 (ACT) with
     bn_stats/bn_aggr reading the staged f16 (DVE), v copies split
     ACT/DVE by parity (pass A is otherwise DVE-bound); then one batched
     rsqrt for all 64 rows via
     exp(-0.5*ln(var+eps)) — Ln/Exp share one ACT table set with the
     attention Exp, so the whole kernel needs ~3 table loads (a per-tile
     Sqrt thrashes 3.6us table reloads against the attention Exps);
     (B) apply LN on DVE (per-partition scale + broadcast bias) into
     [s,t,h] staging, one batched DMA-xbar-transpose per tensor to [h,t,s],
     then a single big ACT op folds LN weight/bias (per-partition scalars
     after the transpose). ALL xbar transposes go on the sync HWDGE ring:
     concurrent transposes on the sync+scalar rings corrupt data on HW.
  3. attention with TRANSPOSED scores (scT[j,i] = k_j . q_i), looping over
     1024-wide query column blocks (i-quarters):
     - per key tile jt (128 keys): scT = matmul(lhsT=kT[:,jt,:],
       rhs=qT columns) into a 2-bank PSUM tile; exp via ACT off PSUM with
       scale=1/sqrt(D) directly into eT [j, i] f16 — already the layout PV
       needs, so the 32MB of per-i-tile exp xbar transposes of the old
       scheme disappear entirely. No max subtraction: scores are ~N(0,1)
       (LN'd q,k), exp stays in f16 range.
     - PV accumulates outT[h, i] += v16[:,jt,:]^T @ eT over all 32 key
       tiles (v is the stationary matmul operand, eT streams 512-col
       chunks), and a parallel ones-column matmul accumulates the softmax
       denominators rs[1, i] — 6 wide 512-col matmuls per key tile instead
       of the old scheme's 32 tiny 129-col PV matmuls per query tile.
     - drain: outT -> f16 SBUF (DVE), one xbar transpose per i-quarter
       back to [i, h]; rs row is PE-transposed (matmul-with-identity, 32
       [1,128]->[128,1] blocks) into per-partition scalars, reciprocal on
       DVE, then per i-tile normalize (tensor_scalar) and DMA out.
     Software pipeline: QK[jt] is emitted before PV[jt-1] so the PE runs
     the ACT-feeding scores matmul first; sc PSUM pool is double-buffered,
     eT triple-buffered.

All SBUF pools stay open for the whole kernel (no SBUF slot reuse across
phases): SBUF-space reuse attaches release waits to the DMAs that load into
recycled space, and walrus rejects DMAs with more than a couple of sync
waits ("Too many sync wait commands"). Only PSUM pools are scoped.
"""

import math

import numpy as np

from concourse import bacc
import concourse.mybir as mybir
import concourse.tile as tile
from concourse.bass_utils import run_bass_kernel_spmd


F16 = mybir.dt.float16
F32 = mybir.dt.float32
AF = mybir.ActivationFunctionType
ALU = mybir.AluOpType

B, S, D = 8, 4096, 128
P = 128
NT = S // P  # 32 s-tiles
EPS = 1e-5
ISQRT_D = 1.0 / math.sqrt(D)
N_CORES = 8
_ABLATE = set()  # timing-ablation flags, empty in production
IQ = 4  # query-column blocks in the attention loop (1024 cols = 2 banks)


def _ln_param_to_sbuf(nc, pool, dram_ap, tag):
    t = pool.tile([P, 1], F32, tag=tag)
    nc.sync.dma_start(t, dram_ap[:, None])
    return t


def _build_attention(tc, out_d, x_d, w_d, ln_d):
    """Emit the single-core attention program.

    out_d: [S, D] f32 output AP.  x_d: [S, D] f32 input AP.
    w_d: dict q/k/v -> [D, D] f32 weight AP (torch Linear layout: out = x @ W^T).
    ln_d: dict qw/qb/kw/kb -> [D] f32 LN param APs.
    """
    nc = tc.nc

    with (
        tc.tile_pool(name="const", bufs=1) as const,
        tc.tile_pool(name="big", bufs=1) as big,
        tc.tile_pool(name="wtmp", bufs=3) as wtmp,
        tc.tile_pool(name="xload", bufs=1) as xload,
        tc.tile_pool(name="stat", bufs=6) as stat,
        tc.tile_pool(name="attn", bufs=4) as attn,
        tc.tile_pool(name="stage", bufs=2) as stage,
        tc.tile_pool(name="small", bufs=4) as small,
    ):
        # --- weights: load [h,d] f32, cast f16, DMA-xbar-transpose -> W^T f16
        WT = {}
        for name in ("q", "k", "v"):
            w32 = wtmp.tile([P, P], F32, tag=f"w32_{name}")
            nc.sync.dma_start(w32, w_d[name])
            w16 = wtmp.tile([P, P], F16, tag=f"w16_{name}")
            nc.vector.tensor_copy(w16, w32)
            wt = const.tile([P, P], F16, tag=f"wt_{name}")
            nc.sync.dma_start_transpose(wt, w16)
            WT[name] = wt

        # --- x load (cast f32->f16 in the SWDGE DMA), one batched
        # DMA-xbar-transpose: xT[d, t, s] = x16[s, t*128+d]
        xT = big.tile([P, NT, P], F16, tag="xT")  # [d, t, s%128]
        x16 = xload.tile([P, NT, P], F16)  # [s%128, t, d]
        _ab = _ABLATE
        # load and transpose in quarters so the first projections start
        # as soon as the first 8 s-tiles are resident
        x_r = x_d.rearrange("(t p) d -> p t d", p=P)
        x16f = x16.rearrange("p t d -> p (t d)")
        for c in range(4):
            nc.gpsimd.dma_start(x16[:, c * 8:(c + 1) * 8, :],
                                x_r[:, c * 8:(c + 1) * 8, :])
            nc.sync.dma_start_transpose(
                xT[:, c * 8:(c + 1) * 8, :],
                x16f[:, c * 8 * P:(c + 1) * 8 * P])

        # LN params loaded after the x/weight ring traffic: they head-of-line
        # block the sync ring for ~2us if issued first, and aren't needed
        # until pass B
        qnw = _ln_param_to_sbuf(nc, const, ln_d["qw"], "qnw")
        qnb = _ln_param_to_sbuf(nc, const, ln_d["qb"], "qnb")
        knw = _ln_param_to_sbuf(nc, const, ln_d["kw"], "knw")
        knb = _ln_param_to_sbuf(nc, const, ln_d["kb"], "knb")

        # --- projections + layernorm -> qT, kT [h, s] f16; v [s, h] f16
        # Two passes: (A) project q/k/v, stage raw q/k + bn stats; then ONE
        # batched rsqrt for all 64 (tile, tensor) rows via exp(-0.5*ln(v+eps))
        # (a single Ln + Exp keeps ACT table switching to ~2 loads; per-tile
        # Sqrt thrashes table sets against the attention Exp ops); (B) apply
        # LN, transpose, fold ln weight/bias.
        qT = big.tile([P, NT, P], F16, tag="qT")
        kT = big.tile([P, NT, P], F16, tag="kT")
        v16 = big.tile([P, NT, P], F16, tag="v16")
        raw = big.tile([P, 2 * NT, P], F16, tag="raw")  # pre-LN q/k
        mvall = big.tile([P, 2 * NT, 2], F32, tag="mvall")  # (mean, var)
        rstd_all = big.tile([P, 2 * NT], F32, tag="rstd_all")
        nmr_all = big.tile([P, 2 * NT], F32, tag="nmr_all")
        s1q = big.tile([P, NT, P], F16, tag="s1q")
        s1k = big.tile([P, NT, P], F16, tag="s1k")
        qT_pre = big.tile([P, NT, P], F16, tag="qT_pre")
        kT_pre = big.tile([P, NT, P], F16, tag="kT_pre")
        specs = {
            "q": (s1q, qT_pre, qT, qnw, qnb, 0),
            "k": (s1k, kT_pre, kT, knw, knb, 1),
        }

        def emit_ln_half(name, hh):
            # LN apply (DVE) -> xbar transpose (sync ring) -> wb fold (ACT)
            s1all, pre, Tdst, wsb, bsb, koff = specs[name]
            for t in range(hh * 16, (hh + 1) * 16):
                idx = 2 * t + koff
                nc.vector.scalar_tensor_tensor(
                    s1all[:, t, :], in0=raw[:, idx, :],
                    scalar=rstd_all[:, idx:idx + 1],
                    in1=nmr_all[:, idx:idx + 1].to_broadcast([P, P]),
                    op0=ALU.mult, op1=ALU.add)
            s1f = s1all.rearrange("p t h -> p (t h)")
            T2 = Tdst.rearrange("h t s -> h (t s)")
            P2 = pre.rearrange("h t s -> h (t s)")
            nc.sync.dma_start_transpose(
                pre[:, hh * 16:(hh + 1) * 16, :],
                s1f[:, hh * 16 * P:(hh + 1) * 16 * P])
            nc.scalar.activation(
                T2[:, hh * 16 * P:(hh + 1) * 16 * P],
                P2[:, hh * 16 * P:(hh + 1) * 16 * P],
                AF.Identity, scale=wsb, bias=bsb)

        # Projection + LN run in HALVES so half 0's rsqrt/apply/transpose/
        # fold chain overlaps half 1's projections: attention then starts
        # right after half 1's LN instead of serializing the whole chain
        # behind all 32 projections. Half order q,k then k,q: attention
        # consumes kT progressively but needs qT half 0 for its first
        # query block.
        with tc.tile_pool(name="pps", bufs=2, space="PSUM") as pps:
            for hh in range(2 if "ph12" not in _ab else 0):
                for t in range(hh * 16, (hh + 1) * 16):
                    for k, name in enumerate(("q", "k", "v")):
                        ps = pps.tile([P, P], F32, tag=f"p_{name}")
                        nc.tensor.matmul(ps, lhsT=xT[:, t, :], rhs=WT[name],
                                         start=True, stop=True)
                        if name == "v":
                            # split PSUM evacuations across ACT/DVE: pass A
                            # is otherwise DVE-bound, ACT has headroom
                            # (gpsimd tensor_copy from PSUM fails to lower)
                            if t % 2 == 0:
                                nc.vector.tensor_copy(v16[:, t, :], ps)
                            else:
                                nc.scalar.activation(v16[:, t, :], ps,
                                                     AF.Copy)
                        else:
                            idx = 2 * t + k
                            nc.scalar.activation(raw[:, idx, :], ps, AF.Copy)
                            # stats from the staged f16 raw (SBUF read is
                            # cheaper on DVE than PSUM, and the stats then
                            # describe exactly what pass B normalizes)
                            st = stat.tile([P, 6], F32, tag="st")
                            nc.vector.bn_stats(st, raw[:, idx, :])
                            nc.vector.bn_aggr(mvall[:, idx, :], st)
                # rsqrt(v) = exp(-0.5 * ln(v)) batched over this half's 32
                # (tile, tensor) rows; Ln/Exp share one ACT table set with
                # the attention Exp so this costs no extra table loads
                sl = slice(hh * NT, (hh + 1) * NT)
                vare = stat.tile([P, NT], F32, tag=f"vare{hh}")
                nc.vector.tensor_scalar_add(vare, mvall[:, sl, 1], EPS)
                nc.scalar.activation(rstd_all[:, sl], vare, AF.Ln)
                nc.scalar.activation(rstd_all[:, sl], rstd_all[:, sl],
                                     AF.Exp, scale=-0.5)
                nc.vector.scalar_tensor_tensor(
                    nmr_all[:, sl], in0=mvall[:, sl, 0], scalar=-1.0,
                    in1=rstd_all[:, sl], op0=ALU.mult, op1=ALU.mult)
                for name in (("q", "k") if hh == 0 else ("k", "q")):
                    emit_ln_half(name, hh)

        # --- attention (transposed scores: scT[j, i] = k_j . q_i)
        qT2 = qT.rearrange("h t s -> h (t s)")
        ones11 = const.tile([1, 1], F32, tag="ones11")
        nc.vector.memset(ones11, 1.0)
        ones16 = const.tile([P, 1], F16, tag="ones16")
        nc.vector.memset(ones16, 1.0)
        rs_sb = big.tile([1, S], F32, tag="rs_sb")  # softmax denominators
        rsrT = big.tile([P, NT], F32, tag="rsrT")  # 1/rs, [i%128, it]
        oT = big.tile([P, NT, P], F16, tag="oT")  # [i%128, it, h] unnormalized

        IQW = S // IQ  # query columns per block
        TPQ = NT // IQ  # i-tiles per block
        with (
            tc.tile_pool(name="scps", bufs=3, space="PSUM") as scps,
            tc.tile_pool(name="outps", bufs=1, space="PSUM") as outps,
            tc.tile_pool(name="rcp", bufs=2) as rcp,
        ):
            for iq in range(IQ):
                outp = outps.tile([P, IQW], F32, tag="outp")
                # rowsum partials accumulate on DVE (f16 ping-pong): frees
                # 2 PE matmuls per key tile AND the rs PSUM banks, which
                # buy the 3-deep scores pool that hides the QK->exp->free
                # semaphore latency. Partitions collapse via one
                # ones-matmul per block at the end.
                # eight independent accumulate chains (strided by key
                # tile): a single chain is a serial 1.13us-latency
                # dependency; many chains make it throughput-bound AND turn
                # more adds into 2x-mode first-use copies (DVE is the
                # binding engine of the attention loop)
                racc = []
                for c in range(8):
                    rc0 = rcp.tile([P, IQW], F16, tag=f"racc{c}_0")
                    rc1 = rcp.tile([P, IQW], F16, tag=f"racc{c}_1")
                    racc.append([rc0, rc1])

                def emit_qk_exp(jt, iq=iq):
                    sc = scps.tile([P, IQW], F32, tag="sc")
                    if "qk" not in _ab:
                        for h in range(IQW // 512):
                            nc.tensor.matmul(
                                sc[:, h * 512:(h + 1) * 512],
                                lhsT=kT[:, jt, :],
                                rhs=qT2[:, iq * IQW + h * 512:
                                        iq * IQW + (h + 1) * 512],
                                start=True, stop=True)
                    eT = attn.tile([P, IQW], F16, tag="eT")  # [j%128, i]
                    if "exp" not in _ab:
                        nc.scalar.activation(eT, sc, AF.Exp, scale=ISQRT_D)
                    return eT

                def emit_pv(jt, eT, outp=outp, racc=racc):
                    first, last = jt == 0, jt == NT - 1
                    if "pv" not in _ab:
                        for h in range(IQW // 512):
                            nc.tensor.matmul(
                                outp[:, h * 512:(h + 1) * 512],
                                lhsT=v16[:, jt, :],
                                rhs=eT[:, h * 512:(h + 1) * 512],
                                start=first, stop=last)
                    chain, m = racc[jt % 8], jt // 8
                    if m == 0:
                        nc.vector.tensor_copy(chain[0], eT)
                    else:
                        nc.vector.scalar_tensor_tensor(
                            chain[m % 2], in0=chain[(m + 1) % 2],
                            scalar=1.0, in1=eT,
                            op0=ALU.mult, op1=ALU.add)

                # sw-pipeline: QK[jt] ahead of PV[jt-1] in the PE stream so
                # the ACT-feeding scores matmul never queues behind PV
                prev = None
                for jt in range(NT):
                    cur = emit_qk_exp(jt)
                    if prev is not None:
                        emit_pv(jt - 1, prev)
                    prev = cur
                emit_pv(NT - 1, prev)

                # drain this query block: outT -> f16, xbar back to [i, h];
                # rs row -> per-partition scalars via k=1 matmuls
                # (out[m, 0] = rs_row[0, m] * ones[0, 0]) reusing the rsps
                # pool slot, then normalize + write out this block — keeps
                # the output DMA overlapped with the next block's attention
                o16 = stage.tile([P, IQW], F16, tag="o16")
                # ACT copy: DVE is the binding engine in the attention loop
                nc.scalar.activation(o16, outp, AF.Copy)
                nc.sync.dma_start_transpose(
                    oT[:, iq * TPQ:(iq + 1) * TPQ, :], o16)
                # rs partials -> [1, IQW] via ones-matmul, riding the outp
                # PSUM slot (free once o16 is drained)
                # collapse partitions AND combine the chains in one go:
                # 8 accumulating ones-matmuls into the rs row (each chain's
                # last write is slot 1: m = 3 for all chains)
                rsp = outps.tile([1, IQW], F32, tag="outp")
                for c in range(8):
                    for h in range(IQW // 512):
                        nc.tensor.matmul(
                            rsp[:, h * 512:(h + 1) * 512], lhsT=ones16,
                            rhs=racc[c][1][:, h * 512:(h + 1) * 512],
                            start=(c == 0), stop=(c == 7))
                nc.vector.tensor_copy(rs_sb[:, iq * IQW:(iq + 1) * IQW], rsp)
                rsT_ps = outps.tile([P, TPQ], F32, tag="outp")
                for tt in range(TPQ):
                    t = iq * TPQ + tt
                    nc.tensor.matmul(rsT_ps[:, tt:tt + 1],
                                     lhsT=rs_sb[:, t * P:(t + 1) * P],
                                     rhs=ones11, start=True, stop=True)
                nc.vector.reciprocal(rsrT[:, iq * TPQ:(iq + 1) * TPQ], rsT_ps)
                for tt in range(TPQ):
                    t = iq * TPQ + tt
                    osb = small.tile([P, P], F32, tag="osb")
                    nc.vector.tensor_scalar_mul(osb, oT[:, t, :],
                                                rsrT[:, t:t + 1])
                    # alternate rings so the final block's drain isn't
                    # serialized on one DGE ring
                    ring = nc.gpsimd if tt % 2 == 0 else nc.sync
                    ring.dma_start(out_d[t * P:(t + 1) * P, :], osb)


_NC_CACHE = None


def _build():
    global _NC_CACHE
    if _NC_CACHE is not None:
        return _NC_CACHE
    nc = bacc.Bacc("TRN2", target_bir_lowering=False, debug=False)
    x = nc.dram_tensor("x", [S, D], F32, kind="ExternalInput").ap()
    wq = nc.dram_tensor("Wq", [D, D], F32, kind="ExternalInput").ap()
    wk = nc.dram_tensor("Wk", [D, D], F32, kind="ExternalInput").ap()
    wv = nc.dram_tensor("Wv", [D, D], F32, kind="ExternalInput").ap()
    qn_w = nc.dram_tensor("qn_w", [D], F32, kind="ExternalInput").ap()
    qn_b = nc.dram_tensor("qn_b", [D], F32, kind="ExternalInput").ap()
    kn_w = nc.dram_tensor("kn_w", [D], F32, kind="ExternalInput").ap()
    kn_b = nc.dram_tensor("kn_b", [D], F32, kind="ExternalInput").ap()
    out = nc.dram_tensor("out", [S, D], F32, kind="ExternalOutput").ap()
    with tile.TileContext(nc) as tc:
        _build_attention(
            tc, out, x,
            {"q": wq, "k": wk, "v": wv},
            {"qw": qn_w, "qb": qn_b, "kw": kn_w, "kb": kn_b},
        )
    nc.compile()
    _NC_CACHE = nc
    return nc


def kernel(x, Wq, Wk, Wv, qn_w, qn_b, kn_w, kn_b, _run_kwargs=None):
    nc = _build()
    x = np.asarray(x, dtype=np.float32)
    shared = {
        "Wq": np.ascontiguousarray(np.asarray(Wq, np.float32)),
        "Wk": np.ascontiguousarray(np.asarray(Wk, np.float32)),
        "Wv": np.ascontiguousarray(np.asarray(Wv, np.float32)),
        "qn_w": np.ascontiguousarray(np.asarray(qn_w, np.float32)),
        "qn_b": np.ascontiguousarray(np.asarray(qn_b, np.float32)),
        "kn_w": np.ascontiguousarray(np.asarray(kn_w, np.float32)),
        "kn_b": np.ascontiguousarray(np.asarray(kn_b, np.float32)),
    }
    in_maps = [
        {"x": np.ascontiguousarray(x[b]), **shared} for b in range(B)
    ]
    res = run_bass_kernel_spmd(nc, in_maps, core_ids=list(range(N_CORES)),
                               **(_run_kwargs or {}))
    out = np.stack([res.results[b]["out"] for b in range(B)], axis=0)
    if _run_kwargs:
        kernel.last_results = res
    return out.astype(np.float32)



# revision 48
# speedup vs baseline: 1.7077x; 1.3190x over previous
"""Trainium2 Bass kernel for nn_AttentionHead (B=8, S=4096, D=128).

Sharding: data-parallel over the batch dim — 1 batch element per NeuronCore,
8 cores, SPMD (same NEFF, different x slice), weights replicated. No
collectives.

Per-core pipeline (S=4096 seq, D=128 head dim, all-on-chip, f16 compute
with f32 PSUM accumulation; fro rel err vs fp32 reference ~5e-4):
  1. x [4096,128] f32 -> cast-load f16 (SWDGE cast DMA) -> chunked
     DMA-xbar-transposes -> xT [d, s] f16 (chunked so projections start
     after the first quarter)
  2. q/k/v projections: matmul(lhsT=xT s-tile, rhs=W^T) -> PSUM f32.
     Two passes: (A) per tile stage raw q/k to SBUF f16 (ACT) with
     bn_stats/bn_aggr reading the staged f16 (DVE), v copies split
     ACT/DVE by parity (pass A is otherwise DVE-bound); then one batched
     rsqrt for all 64 rows via
     exp(-0.5*ln(var+eps)) — Ln/Exp share one ACT table set with the
     attention Exp, so the whole kernel needs ~3 table loads (a per-tile
     Sqrt thrashes 3.6us table reloads against the attention Exps);
     (B) apply LN on DVE (per-partition scale + broadcast bias) into
     [s,t,h] staging, one batched DMA-xbar-transpose per tensor to [h,t,s],
     then a single big ACT op folds LN weight/bias (per-partition scalars
     after the transpose). ALL xbar transposes go on the sync HWDGE ring:
     concurrent transposes on the sync+scalar rings corrupt data on HW.
  3. attention with TRANSPOSED scores (scT[j,i] = k_j . q_i), looping over
     1024-wide query column blocks (i-quarters):
     - per key tile jt (128 keys): scT = matmul(lhsT=kT[:,jt,:],
       rhs=qT columns) into a 2-bank PSUM tile; exp via ACT off PSUM with
       scale=1/sqrt(D) directly into eT [j, i] f16 — already the layout PV
       needs, so the 32MB of per-i-tile exp xbar transposes of the old
       scheme disappear entirely. No max subtraction: scores are ~N(0,1)
       (LN'd q,k), exp stays in f16 range.
     - PV accumulates outT[h, i] += v16[:,jt,:]^T @ eT over all 32 key
       tiles (v is the stationary matmul operand, eT streams 512-col
       chunks), and a parallel ones-column matmul accumulates the softmax
       denominators rs[1, i] — 6 wide 512-col matmuls per key tile instead
       of the old scheme's 32 tiny 129-col PV matmuls per query tile.
     - drain: outT -> f16 SBUF (DVE), one xbar transpose per i-quarter
       back to [i, h]; rs row is PE-transposed (matmul-with-identity, 32
       [1,128]->[128,1] blocks) into per-partition scalars, reciprocal on
       DVE, then per i-tile normalize (tensor_scalar) and DMA out.
     Software pipeline: QK[jt] is emitted before PV[jt-1] so the PE runs
     the ACT-feeding scores matmul first; sc PSUM pool is double-buffered,
     eT triple-buffered.

All SBUF pools stay open for the whole kernel (no SBUF slot reuse across
phases): SBUF-space reuse attaches release waits to the DMAs that load into
recycled space, and walrus rejects DMAs with more than a couple of sync
waits ("Too many sync wait commands"). Only PSUM pools are scoped.
"""

import math

import numpy as np

from concourse import bacc
import concourse.mybir as mybir
import concourse.tile as tile
from concourse.bass_utils import run_bass_kernel_spmd


F16 = mybir.dt.float16
F32 = mybir.dt.float32
AF = mybir.ActivationFunctionType
ALU = mybir.AluOpType

B, S, D = 8, 4096, 128
P = 128
NT = S // P  # 32 s-tiles
EPS = 1e-5
ISQRT_D = 1.0 / math.sqrt(D)
N_CORES = 8
_ABLATE = set()  # timing-ablation flags, empty in production
IQ = 4  # query-column blocks in the attention loop (1024 cols = 2 banks)


def _ln_param_to_sbuf(nc, pool, dram_ap, tag):
    t = pool.tile([P, 1], F32, tag=tag)
    nc.sync.dma_start(t, dram_ap[:, None])
    return t


def _build_attention(tc, out_d, x_d, w_d, ln_d):
    """Emit the single-core attention program.

    out_d: [S, D] f32 output AP.  x_d: [S, D] f32 input AP.
    w_d: dict q/k/v -> [D, D] f32 weight AP (torch Linear layout: out = x @ W^T).
    ln_d: dict qw/qb/kw/kb -> [D] f32 LN param APs.
    """
    nc = tc.nc

    with (
        tc.tile_pool(name="const", bufs=1) as const,
        tc.tile_pool(name="big", bufs=1) as big,
        tc.tile_pool(name="wtmp", bufs=3) as wtmp,
        tc.tile_pool(name="xload", bufs=1) as xload,
        tc.tile_pool(name="stat", bufs=6) as stat,
        tc.tile_pool(name="attn", bufs=6) as attn,
        tc.tile_pool(name="stage", bufs=2) as stage,
        tc.tile_pool(name="small", bufs=4) as small,
    ):
        # --- weights: load [h,d] f32, cast f16, DMA-xbar-transpose -> W^T f16
        WT = {}
        for name in ("q", "k", "v"):
            w32 = wtmp.tile([P, P], F32, tag=f"w32_{name}")
            nc.sync.dma_start(w32, w_d[name])
            w16 = wtmp.tile([P, P], F16, tag=f"w16_{name}")
            nc.vector.tensor_copy(w16, w32)
            wt = const.tile([P, P], F16, tag=f"wt_{name}")
            nc.sync.dma_start_transpose(wt, w16)
            WT[name] = wt

        # --- x load (cast f32->f16 in the SWDGE DMA), one batched
        # DMA-xbar-transpose: xT[d, t, s] = x16[s, t*128+d]
        xT = big.tile([P, NT, P], F16, tag="xT")  # [d, t, s%128]
        x16 = xload.tile([P, NT, P], F16)  # [s%128, t, d]
        _ab = _ABLATE
        # load and transpose in quarters so the first projections start
        # as soon as the first 8 s-tiles are resident
        x_r = x_d.rearrange("(t p) d -> p t d", p=P)
        x16f = x16.rearrange("p t d -> p (t d)")
        for c in range(4):
            nc.gpsimd.dma_start(x16[:, c * 8:(c + 1) * 8, :],
                                x_r[:, c * 8:(c + 1) * 8, :])
            nc.sync.dma_start_transpose(
                xT[:, c * 8:(c + 1) * 8, :],
                x16f[:, c * 8 * P:(c + 1) * 8 * P])

        # LN params loaded after the x/weight ring traffic: they head-of-line
        # block the sync ring for ~2us if issued first, and aren't needed
        # until pass B
        qnw = _ln_param_to_sbuf(nc, const, ln_d["qw"], "qnw")
        qnb = _ln_param_to_sbuf(nc, const, ln_d["qb"], "qnb")
        knw = _ln_param_to_sbuf(nc, const, ln_d["kw"], "knw")
        knb = _ln_param_to_sbuf(nc, const, ln_d["kb"], "knb")

        # --- projections + layernorm -> qT, kT [h, s] f16; v [s, h] f16
        # Two passes: (A) project q/k/v, stage raw q/k + bn stats; then ONE
        # batched rsqrt for all 64 (tile, tensor) rows via exp(-0.5*ln(v+eps))
        # (a single Ln + Exp keeps ACT table switching to ~2 loads; per-tile
        # Sqrt thrashes table sets against the attention Exp ops); (B) apply
        # LN, transpose, fold ln weight/bias.
        qT = big.tile([P, NT, P], F16, tag="qT")
        kT = big.tile([P, NT, P], F16, tag="kT")
        v16 = big.tile([P, NT, P], F16, tag="v16")
        raw = big.tile([P, 2 * NT, P], F16, tag="raw")  # pre-LN q/k
        mvall = big.tile([P, 2 * NT, 2], F32, tag="mvall")  # (mean, var)
        rstd_all = big.tile([P, 2 * NT], F32, tag="rstd_all")
        nmr_all = big.tile([P, 2 * NT], F32, tag="nmr_all")
        s1q = big.tile([P, NT, P], F16, tag="s1q")
        s1k = big.tile([P, NT, P], F16, tag="s1k")
        qT_pre = big.tile([P, NT, P], F16, tag="qT_pre")
        kT_pre = big.tile([P, NT, P], F16, tag="kT_pre")
        specs = {
            "q": (s1q, qT_pre, qT, qnw, qnb, 0),
            "k": (s1k, kT_pre, kT, knw, knb, 1),
        }

        def emit_ln_half(name, hh):
            # LN apply (DVE) -> xbar transpose (sync ring) -> wb fold (ACT)
            s1all, pre, Tdst, wsb, bsb, koff = specs[name]
            for t in range(hh * 16, (hh + 1) * 16):
                idx = 2 * t + koff
                nc.vector.scalar_tensor_tensor(
                    s1all[:, t, :], in0=raw[:, idx, :],
                    scalar=rstd_all[:, idx:idx + 1],
                    in1=nmr_all[:, idx:idx + 1].to_broadcast([P, P]),
                    op0=ALU.mult, op1=ALU.add)
            s1f = s1all.rearrange("p t h -> p (t h)")
            T2 = Tdst.rearrange("h t s -> h (t s)")
            P2 = pre.rearrange("h t s -> h (t s)")
            nc.sync.dma_start_transpose(
                pre[:, hh * 16:(hh + 1) * 16, :],
                s1f[:, hh * 16 * P:(hh + 1) * 16 * P])
            nc.scalar.activation(
                T2[:, hh * 16 * P:(hh + 1) * 16 * P],
                P2[:, hh * 16 * P:(hh + 1) * 16 * P],
                AF.Identity, scale=wsb, bias=bsb)

        # Projection + LN run in HALVES so half 0's rsqrt/apply/transpose/
        # fold chain overlaps half 1's projections: attention then starts
        # right after half 1's LN instead of serializing the whole chain
        # behind all 32 projections. Half order q,k then k,q: attention
        # consumes kT progressively but needs qT half 0 for its first
        # query block.
        with tc.tile_pool(name="pps", bufs=2, space="PSUM") as pps:
            for hh in range(2 if "ph12" not in _ab else 0):
                for t in range(hh * 16, (hh + 1) * 16):
                    for k, name in enumerate(("q", "k", "v")):
                        ps = pps.tile([P, P], F32, tag=f"p_{name}")
                        nc.tensor.matmul(ps, lhsT=xT[:, t, :], rhs=WT[name],
                                         start=True, stop=True)
                        if name == "v":
                            # split PSUM evacuations across ACT/DVE: pass A
                            # is otherwise DVE-bound, ACT has headroom
                            # (gpsimd tensor_copy from PSUM fails to lower)
                            if t % 2 == 0:
                                nc.vector.tensor_copy(v16[:, t, :], ps)
                            else:
                                nc.scalar.activation(v16[:, t, :], ps,
                                                     AF.Copy)
                        else:
                            idx = 2 * t + k
                            nc.scalar.activation(raw[:, idx, :], ps, AF.Copy)
                            # stats from the staged f16 raw (SBUF read is
                            # cheaper on DVE than PSUM, and the stats then
                            # describe exactly what pass B normalizes)
                            st = stat.tile([P, 6], F32, tag="st")
                            nc.vector.bn_stats(st, raw[:, idx, :])
                            nc.vector.bn_aggr(mvall[:, idx, :], st)
                # rsqrt(v) = exp(-0.5 * ln(v)) batched over this half's 32
                # (tile, tensor) rows; Ln/Exp share one ACT table set with
                # the attention Exp so this costs no extra table loads
                sl = slice(hh * NT, (hh + 1) * NT)
                vare = stat.tile([P, NT], F32, tag=f"vare{hh}")
                nc.vector.tensor_scalar_add(vare, mvall[:, sl, 1], EPS)
                nc.scalar.activation(rstd_all[:, sl], vare, AF.Ln)
                nc.scalar.activation(rstd_all[:, sl], rstd_all[:, sl],
                                     AF.Exp, scale=-0.5)
                nc.vector.scalar_tensor_tensor(
                    nmr_all[:, sl], in0=mvall[:, sl, 0], scalar=-1.0,
                    in1=rstd_all[:, sl], op0=ALU.mult, op1=ALU.mult)
                for name in (("q", "k") if hh == 0 else ("k", "q")):
                    emit_ln_half(name, hh)

        # --- attention (transposed scores: scT[j, i] = k_j . q_i)
        qT2 = qT.rearrange("h t s -> h (t s)")
        ones11 = const.tile([1, 1], F32, tag="ones11")
        nc.vector.memset(ones11, 1.0)
        ones16 = const.tile([P, 1], F16, tag="ones16")
        nc.vector.memset(ones16, 1.0)
        rs_sb = big.tile([1, S], F32, tag="rs_sb")  # softmax denominators
        rsrT = big.tile([P, NT], F32, tag="rsrT")  # 1/rs, [i%128, it]
        oT = big.tile([P, NT, P], F16, tag="oT")  # [i%128, it, h] unnormalized

        IQW = S // IQ  # query columns per block
        TPQ = NT // IQ  # i-tiles per block
        with (
            tc.tile_pool(name="scps", bufs=3, space="PSUM") as scps,
            tc.tile_pool(name="outps", bufs=1, space="PSUM") as outps,
            tc.tile_pool(name="rcp", bufs=2) as rcp,
        ):
            for iq in range(IQ):
                outp = outps.tile([P, IQW], F32, tag="outp")
                # rowsum partials accumulate on DVE (f16 ping-pong): frees
                # 2 PE matmuls per key tile AND the rs PSUM banks, which
                # buy the 3-deep scores pool that hides the QK->exp->free
                # semaphore latency. Partitions collapse via one
                # ones-matmul per block at the end.
                # eight independent accumulate chains (strided by key
                # tile): a single chain is a serial 1.13us-latency
                # dependency; many chains make it throughput-bound AND turn
                # more adds into 2x-mode first-use copies (DVE is the
                # binding engine of the attention loop)
                racc = []
                for c in range(8):
                    rc0 = rcp.tile([P, IQW], F16, tag=f"racc{c}_0")
                    rc1 = rcp.tile([P, IQW], F16, tag=f"racc{c}_1")
                    racc.append([rc0, rc1])

                def emit_qk_exp(jt, iq=iq):
                    sc = scps.tile([P, IQW], F32, tag="sc")
                    if "qk" not in _ab:
                        for h in range(IQW // 512):
                            nc.tensor.matmul(
                                sc[:, h * 512:(h + 1) * 512],
                                lhsT=kT[:, jt, :],
                                rhs=qT2[:, iq * IQW + h * 512:
                                        iq * IQW + (h + 1) * 512],
                                start=True, stop=True)
                    eT = attn.tile([P, IQW], F16, tag="eT")  # [j%128, i]
                    if "exp" not in _ab:
                        nc.scalar.activation(eT, sc, AF.Exp, scale=ISQRT_D)
                    return eT

                def emit_pv(jt, eT, outp=outp, racc=racc):
                    first, last = jt == 0, jt == NT - 1
                    if "pv" not in _ab:
                        for h in range(IQW // 512):
                            nc.tensor.matmul(
                                outp[:, h * 512:(h + 1) * 512],
                                lhsT=v16[:, jt, :],
                                rhs=eT[:, h * 512:(h + 1) * 512],
                                start=first, stop=last)
                    chain, m = racc[jt % 8], jt // 8
                    if m == 0:
                        nc.vector.tensor_copy(chain[0], eT)
                    else:
                        nc.vector.scalar_tensor_tensor(
                            chain[m % 2], in0=chain[(m + 1) % 2],
                            scalar=1.0, in1=eT,
                            op0=ALU.mult, op1=ALU.add)

                # sw-pipeline: QK[jt] ahead of PV[jt-1] in the PE stream so
                # the ACT-feeding scores matmul never queues behind PV
                prev = None
                for jt in range(NT):
                    cur = emit_qk_exp(jt)
                    if prev is not None:
                        emit_pv(jt - 1, prev)
                    prev = cur
                emit_pv(NT - 1, prev)

                # drain this query block: outT -> f16, xbar back to [i, h];
                # rs row -> per-partition scalars via k=1 matmuls
                # (out[m, 0] = rs_row[0, m] * ones[0, 0]) reusing the rsps
                # pool slot, then normalize + write out this block — keeps
                # the output DMA overlapped with the next block's attention
                o16 = stage.tile([P, IQW], F16, tag="o16")
                # ACT copy: DVE is the binding engine in the attention loop
                nc.scalar.activation(o16, outp, AF.Copy)
                nc.sync.dma_start_transpose(
                    oT[:, iq * TPQ:(iq + 1) * TPQ, :], o16)
                # rs partials -> [1, IQW] via ones-matmul, riding the outp
                # PSUM slot (free once o16 is drained)
                # collapse partitions AND combine the chains in one go:
                # 8 accumulating ones-matmuls into the rs row (each chain's
                # last write is slot 1: m = 3 for all chains)
                # last block: the rs row rides a free scores slot instead
                # of the outp slot, so the ones-matmuls need not wait for
                # the o16 drain — shortens the serial tail (safe only on
                # the final block: there is no next QK to stall)
                if iq == IQ - 1:
                    rsp = scps.tile([1, IQW], F32, tag="sc")
                else:
                    rsp = outps.tile([1, IQW], F32, tag="outp")
                for c in range(8):
                    for h in range(IQW // 512):
                        nc.tensor.matmul(
                            rsp[:, h * 512:(h + 1) * 512], lhsT=ones16,
                            rhs=racc[c][1][:, h * 512:(h + 1) * 512],
                            start=(c == 0), stop=(c == 7))
                nc.vector.tensor_copy(rs_sb[:, iq * IQW:(iq + 1) * IQW], rsp)
                if iq == IQ - 1:
                    rsT_ps = scps.tile([P, TPQ], F32, tag="sc")
                else:
                    rsT_ps = outps.tile([P, TPQ], F32, tag="outp")
                for tt in range(TPQ):
                    t = iq * TPQ + tt
                    nc.tensor.matmul(rsT_ps[:, tt:tt + 1],
                                     lhsT=rs_sb[:, t * P:(t + 1) * P],
                                     rhs=ones11, start=True, stop=True)
                nc.vector.reciprocal(rsrT[:, iq * TPQ:(iq + 1) * TPQ], rsT_ps)
                for tt in range(TPQ):
                    t = iq * TPQ + tt
                    osb = small.tile([P, P], F32, tag="osb")
                    nc.vector.tensor_scalar_mul(osb, oT[:, t, :],
                                                rsrT[:, t:t + 1])
                    # alternate rings so the final block's drain isn't
                    # serialized on one DGE ring
                    ring = nc.gpsimd if tt % 2 == 0 else nc.sync
                    ring.dma_start(out_d[t * P:(t + 1) * P, :], osb)


_NC_CACHE = None


def _build():
    global _NC_CACHE
    if _NC_CACHE is not None:
        return _NC_CACHE
    nc = bacc.Bacc("TRN2", target_bir_lowering=False, debug=False)
    x = nc.dram_tensor("x", [S, D], F32, kind="ExternalInput").ap()
    wq = nc.dram_tensor("Wq", [D, D], F32, kind="ExternalInput").ap()
    wk = nc.dram_tensor("Wk", [D, D], F32, kind="ExternalInput").ap()
    wv = nc.dram_tensor("Wv", [D, D], F32, kind="ExternalInput").ap()
    qn_w = nc.dram_tensor("qn_w", [D], F32, kind="ExternalInput").ap()
    qn_b = nc.dram_tensor("qn_b", [D], F32, kind="ExternalInput").ap()
    kn_w = nc.dram_tensor("kn_w", [D], F32, kind="ExternalInput").ap()
    kn_b = nc.dram_tensor("kn_b", [D], F32, kind="ExternalInput").ap()
    out = nc.dram_tensor("out", [S, D], F32, kind="ExternalOutput").ap()
    with tile.TileContext(nc) as tc:
        _build_attention(
            tc, out, x,
            {"q": wq, "k": wk, "v": wv},
            {"qw": qn_w, "qb": qn_b, "kw": kn_w, "kb": kn_b},
        )
    nc.compile()
    _NC_CACHE = nc
    return nc


def kernel(x, Wq, Wk, Wv, qn_w, qn_b, kn_w, kn_b, _run_kwargs=None):
    nc = _build()
    x = np.asarray(x, dtype=np.float32)
    shared = {
        "Wq": np.ascontiguousarray(np.asarray(Wq, np.float32)),
        "Wk": np.ascontiguousarray(np.asarray(Wk, np.float32)),
        "Wv": np.ascontiguousarray(np.asarray(Wv, np.float32)),
        "qn_w": np.ascontiguousarray(np.asarray(qn_w, np.float32)),
        "qn_b": np.ascontiguousarray(np.asarray(qn_b, np.float32)),
        "kn_w": np.ascontiguousarray(np.asarray(kn_w, np.float32)),
        "kn_b": np.ascontiguousarray(np.asarray(kn_b, np.float32)),
    }
    in_maps = [
        {"x": np.ascontiguousarray(x[b]), **shared} for b in range(B)
    ]
    res = run_bass_kernel_spmd(nc, in_maps, core_ids=list(range(N_CORES)),
                               **(_run_kwargs or {}))
    out = np.stack([res.results[b]["out"] for b in range(B)], axis=0)
    if _run_kwargs:
        kernel.last_results = res
    return out.astype(np.float32)

